# revision 19
# baseline (speedup 1.0000x reference)
"""Trainium2 Bass kernel for nn_BAC_15152644620305 (v3).

Per batch element (1 per NeuronCore, 8 cores):
  p_dense = relu(p @ W1 + b1); q_dense = relu(q @ W2 + b2)
  A = (p_dense @ q_dense.T) / sqrt(600)
  passage_aligned = softmax_rows(A) @ passage ; query_aligned = softmax_cols(A).T @ query
  6 factorization-machine heads on {concat, diff, mul} pairs -> [L, 3] x 2 outputs.

v3 changes over v2 (cost-model-driven):
  - All activation layouts are prepared host-side and DMA'd in final form:
    xT bf16 chunks, xTp fp8 dense pairs, natural fp8 pair tiles with the
    ones column, fp8 weight pairs and FM stationaries.  This deletes the
    whole on-device transpose/convert phase (PE transposes, PSUM
    evictions, fp8 conversions, nats builds, most memsets).
  - AB projections accumulate the x-side and b-side contributions into the
    same PSUM rows (TA/lin sums formed in f32 by the PE), so the DVE
    TA-add disappears and S shrinks to 29 rows; TA^2 is squared straight
    from PSUM on ACT.
  - One AB psum tile holds all 4 nx groups at 32-aligned tile positions.
  - Engine assignment rebalanced: b^2 planes on Pool, xb/xb^2 on DVE,
    x^2 on ACT; dense relu evictions cycle ACT/DVE/Pool.
"""
import numpy as np

L_FULL = 2048
D = 600
U = 300
KFM = 5
N_CORES = 8
SCALE = float(1.0 / np.sqrt(np.float32(D)))
USC = 256.0           # fp8 pre-scale for the u = sum V^2 stationaries

DCH = [(0, 128), (128, 128), (256, 128), (384, 128), (512, 88)]   # D chunks
ONES_COL = 96         # ones column within the 128-wide natural tail tile
ONES_ROW = 96         # denominator row in the pass-A psum
NATW = 640


def _emit(nc, L):
    import concourse.bass as bass
    import concourse.mybir as mybir
    import concourse.tile as tile
    from concourse.masks import make_identity
    from contextlib import ExitStack

    f32 = mybir.dt.float32
    bf16 = mybir.dt.bfloat16
    fp8 = mybir.dt.float8e4
    AF = mybir.ActivationFunctionType
    ALU = mybir.AluOpType
    ds = bass.ds
    DR = mybir.MatmulPerfMode.DoubleRow

    LT = L // 128               # 16 l-tiles
    NP = LT // 2                # 8 pair tiles
    NH = L // 1024              # halves (2)

    x8_d = nc.dram_tensor("x8", [2, 3, 128, 2, L], fp8, kind="ExternalInput")
    xt_d = nc.dram_tensor("xt", [2, 5, 128, L], bf16, kind="ExternalInput")
    nat_d = nc.dram_tensor("natp", [2, NP, 128, 2, NATW], fp8,
                           kind="ExternalInput")
    wp_d = nc.dram_tensor("wpair8", [3, 128, 2, 1024], fp8,
                          kind="ExternalInput")
    pa_d = nc.dram_tensor("pastat", [10, 128, 96], bf16, kind="ExternalInput")
    pbc_d = nc.dram_tensor("pbcstat", [10, 128, 2, 64], fp8,
                           kind="ExternalInput")
    c2_d = nc.dram_tensor("comb2", [128, 6], bf16, kind="ExternalInput")
    bp_d = nc.dram_tensor("biasp", [128, 6], f32, kind="ExternalInput")
    w0_d = nc.dram_tensor("w0col", [3, 2], f32, kind="ExternalInput")
    out_d = nc.dram_tensor("out", [2, 3, L], f32, kind="ExternalOutput")

    with tile.TileContext(nc) as tc, ExitStack() as ctx:
        const = ctx.enter_context(tc.tile_pool(name="const", bufs=1))
        big = ctx.enter_context(tc.tile_pool(name="big", bufs=1))
        ps = ctx.enter_context(tc.tile_pool(name="ps", bufs=1, space="PSUM"))
        e1_cm = tc.tile_pool(name="e1p", bufs=1, side="right")
        e1p = e1_cm.__enter__()
        x8_cm = tc.tile_pool(name="x8p", bufs=1, side="right")
        x8p = x8_cm.__enter__()

        def acc_t(name="acc"):
            return ps.tile([128, 1024], f32, tag="acc", name=name, bufs=2)

        def fmp_t(name="fmp"):
            return ps.tile([128, 512], f32, tag="fmp", name=name, bufs=4)

        # ---------------- constants / weights (DMA, pre-formatted) --------
        onesb = const.tile([128, 128], bf16, tag="onesb")
        nc.vector.memset(onesb[:], 1.0)
        zerob = const.tile([128, 512], bf16, tag="zerob")
        nc.vector.memset(zerob[:], 0.0)
        w0sb = const.tile([3, 2], f32, tag="w0sb")
        nc.sync.dma_start(w0sb[:], w0_d[:])
        bsb = const.tile([128, 6], f32, tag="bsb")
        nc.sync.dma_start(bsb[:], bp_d[:])
        cb2 = const.tile([128, 6], bf16, tag="cb2")
        nc.sync.dma_start(cb2[:], c2_d[:])
        Wp8 = x8p.tile([128, 3 * 2 * 1024], fp8, tag="Wp8")
        nc.sync.dma_start(
            Wp8[:].rearrange("p (c j u) -> p c j u", c=3, j=2),
            wp_d[:].rearrange("c p j u -> p c j u"))
        Wp8v = Wp8[:].rearrange("p (c j u) -> p c j u", c=3, j=2)
        PAst = const.tile([128, 10 * 96], bf16, tag="PAst")
        nc.sync.dma_start(
            PAst[:].rearrange("p (t c) -> p t c", t=10),
            pa_d[:].rearrange("t p c -> p t c"))
        pa_stat = [[PAst[:, ds((s * 5 + k) * 96, 96)] for k in range(5)]
                   for s in range(2)]
        PBCst = const.tile([128, 10 * 2 * 64], fp8, tag="PBCst")
        nc.sync.dma_start(
            PBCst[:].rearrange("p (t j c) -> p t j c", t=10, j=2),
            pbc_d[:].rearrange("t p j c -> p t j c"))
        PBCv = PBCst[:].rearrange("p (t j c) -> p t j c", t=10, j=2)

        # activation layouts (host-prepped); DMA order sets arrival order
        x8 = [[x8p.tile([128 if pc < 2 else 64, 2, L], fp8,
                        tag=f"x8_{t}_{pc}", name=f"x8_{t}_{pc}")
               for pc in range(3)] for t in range(2)]
        for t in range(2):
            for pc in range(3):
                pp = 128 if pc < 2 else 64
                nc.sync.dma_start(x8[t][pc][:], x8_d[t, pc, 0:pp])
        xT = [[big.tile([128, L], bf16, tag=f"xT{t}_{k}", name=f"xT{t}_{k}")
               for k in range(5)] for t in range(2)]
        for k in range(5):      # t=0 chunks early (PB0 prebuild fills idle)
            dcnt = DCH[k][1]
            nc.scalar.dma_start(xT[0][k][:dcnt, :], xt_d[0, k, 0:dcnt])
        natf = [big.tile([128, NP * 2 * NATW], fp8, tag=f"natf{t}",
                         name=f"natf{t}") for t in range(2)]
        natv = [natf[t][:].rearrange("p (i j c) -> p i j c", i=NP, j=2)
                for t in range(2)]
        for t in range(2):
            nc.scalar.dma_start(
                natv[t],
                nat_d[t].rearrange("i p j c -> p i j c"))
        for k in range(5):
            dcnt = DCH[k][1]
            nc.scalar.dma_start(xT[1][k][:dcnt, :], xt_d[1, k, 0:dcnt])

        # ---------------- dense ----------------
        dTP = [big.tile([128, 2, L], fp8, tag=f"dTP{t}", name=f"dTP{t}")
               for t in range(2)]
        dT2p = [big.tile([32, 2, L], fp8, tag=f"dT2p{t}", name=f"dT2p{t}")
                for t in range(2)]
        for t in range(2):
            nc.gpsimd.memset(dT2p[t][:, 1, :], 0.0)

        def relu_ev(dst, src, bias, eng):
            if eng == "act":
                nc.scalar.activation(dst, src, AF.Relu, bias=bias)
            elif eng == "dve":
                nc.vector.scalar_tensor_tensor(
                    dst, src, bias, zerob[:src.shape[0], :],
                    op0=ALU.add, op1=ALU.max)
            else:
                nc.gpsimd.scalar_tensor_tensor(
                    dst, src, bias, zerob[:src.shape[0], :],
                    op0=ALU.add, op1=ALU.max)

        # Pool/GPSIMD cannot read PSUM, so evictions go ACT/DVE only
        ev_cycle = ["act", "dve"]
        ev_i = 0
        for t in range(2):
            for m, (uoff, ucnt) in enumerate([(0, 128), (128, 128),
                                              (256, 44)]):
                for sx in range(4):
                    sl = ds(sx * 512, 512)
                    accd = fmp_t(name="accd")
                    for pc in range(3):
                        pp = 128 if pc < 2 else 64
                        nc.tensor.matmul(
                            accd[:ucnt, :],
                            Wp8v[:pp, pc, :, ds(t * U + uoff, ucnt)],
                            x8[t][pc][:pp, :, sl],
                            start=(pc == 0), stop=(pc == 2), perf_mode=DR)
                    if m < 2:
                        relu_ev(dTP[t][:, m, sl], accd[:ucnt, :],
                                bsb[:ucnt, t * 3 + m: t * 3 + m + 1],
                                ev_cycle[ev_i % 2])
                        ev_i += 1
                    else:
                        relu_ev(dT2p[t][0:32, 0, sl], accd[0:32, :],
                                bsb[0:32, t * 3 + m: t * 3 + m + 1],
                                ev_cycle[ev_i % 2])
                        ev_i += 1
                        # partition-offset mismatch: STT requires same start
                        # partitions, so this one must be ACT
                        relu_ev(dT2p[t][0:12, 1, sl], accd[32:44, :],
                                bsb[32:44, t * 3 + m: t * 3 + m + 1],
                                "act")

        # x8 only feeds the dense matmuls; free its SBUF before the big
        # E1 allocation
        x8_cm.__exit__(None, None, None)

        # ---------------- affinity -> E (both layouts) ----------
        def e_tiles(pool, tag):
            return [pool.tile([128, 2, L], fp8, tag=f"E{tag}_{pi}",
                              name=f"E{tag}_{pi}") for pi in range(NP)]

        def emit_e_unit(a, b, E, i, h):
            """One (l-tile, half) of E = exp(SCALE * dense_a.T @ dense_b)."""
            e = E[i // 2]
            ej = i % 2
            isl = ds(i * 128, 128)
            acc = acc_t(name="eacc")
            for sx in range(2):
                nsl = ds(h * 1024 + sx * 512, 512)
                asl = ds(sx * 512, 512)
                nc.tensor.matmul(acc[:, asl], dTP[a][:, :, isl],
                                 dTP[b][:, :, nsl],
                                 start=True, stop=False, perf_mode=DR)
                nc.tensor.matmul(acc[:, asl], dT2p[a][:, :, isl],
                                 dT2p[b][:, :, nsl],
                                 start=False, stop=True, perf_mode=DR)
            nc.scalar.activation(e[:, ej, ds(h * 1024, 1024)],
                                 acc[:, :], AF.Exp, scale=SCALE)

        # ---------------- aligned + FM per side ----------------
        def aligned_T(s, E, side_tag, hook=None, r_on_act=True,
                      psa_on_acc=False):
            """alT[k] [d, L] bf16 = normalized aligned.T."""
            # pass A: d 512:600 + ones row
            if psa_on_acc:
                psAt = [acc_t(name="psA") for _ in range(2)]
                psA = [psAt[nx // 2][:, ds((nx % 2) * 512, 512)]
                       for nx in range(4)]
            else:
                psA = [fmp_t(name="psA") for _ in range(4)]
            for pi in range(NP):
                ntl = natv[s][:, pi, :, 512:NATW]
                for nx in range(4):
                    nc.tensor.matmul(psA[nx][:, :],
                                     ntl,
                                     E[pi][:, :, ds(nx * 512, 512)],
                                     start=(pi == 0), stop=(pi == NP - 1),
                                     perf_mode=DR)
            # R chain
            R = big.tile([128, L], bf16, tag="R", name=f"R{side_tag}")
            for h in range(NH):
                rr = rp.tile([128, 1024], f32, tag="rr", name="rr")
                rrb = rp.tile([128, 1024], bf16, tag="rrb", name="rrb")
                for sx in range(2):
                    nc.vector.reciprocal(
                        rr[96:97, ds(sx * 512, 512)],
                        psA[h * 2 + sx][ONES_ROW:ONES_ROW + 1, :])
                nc.vector.tensor_copy(rrb[96:97, :], rr[96:97, :])
                # bc must come from the OTHER psum ring than psA (psA slots
                # are all live until the alT4 eviction, which needs R)
                if psa_on_acc:
                    for sx in range(2):
                        bcx = fmp_t(name="bc")
                        nc.tensor.matmul(bcx[:, :], onesb[96:97, 0:128],
                                         rrb[96:97, ds(sx * 512, 512)],
                                         start=True, stop=True,
                                         tile_position=(96, 0))
                        if r_on_act:
                            nc.scalar.copy(
                                R[:, ds(h * 1024 + sx * 512, 512)], bcx[:, :])
                        else:
                            nc.vector.tensor_copy(
                                R[:, ds(h * 1024 + sx * 512, 512)], bcx[:, :])
                else:
                    bc = acc_t(name="bc")
                    for sx in range(2):
                        nc.tensor.matmul(bc[:, ds(sx * 512, 512)],
                                         onesb[96:97, 0:128],
                                         rrb[96:97, ds(sx * 512, 512)],
                                         start=True, stop=True,
                                         tile_position=(96, 0))
                    if r_on_act:
                        nc.scalar.copy(R[:, ds(h * 1024, 1024)], bc[:, :])
                    else:
                        nc.vector.tensor_copy(R[:, ds(h * 1024, 1024)],
                                              bc[:, :])
            alT = [alp.tile([128, L], bf16, tag=f"alT{k}",
                            name=f"alT{side_tag}{k}") for k in range(5)]
            # evict pass A (d-chunk 4)
            for nx in range(4):
                nsl = ds(nx * 512, 512)
                nc.vector.tensor_mul(alT[4][0:88, nsl], psA[nx][0:88, :],
                                     R[0:88, nsl])
            # passes m=0..3 (hook interleaves independent PE work)
            for m in range(4):
                for h in range(NH):
                    acc = acc_t(name="alacc")
                    for pi in range(NP):
                        nat = natv[s][:, pi, :, ds(m * 128, 128)]
                        for sx in range(2):
                            asl = ds(sx * 512, 512)
                            nsl = ds(h * 1024 + sx * 512, 512)
                            nc.tensor.matmul(acc[:, asl],
                                             nat,
                                             E[pi][:, :, nsl],
                                             start=(pi == 0),
                                             stop=(pi == NP - 1),
                                             perf_mode=DR)
                    hsl = ds(h * 1024, 1024)
                    nc.vector.tensor_mul(alT[m][:, hsl], acc[:, :], R[:, hsl])
                if hook is not None:
                    hook(m)
            return alT

        def prebuild_b2(s, xTs, eng, nk=4):
            """Pair-b tiles (b^2, xb^2) with the b^2 row built early."""
            PBt = []
            for k, (doff, dcnt) in enumerate(DCH[:nk]):
                PB = fmbb.tile([128, 2, L], fp8, tag="PBb", name=f"PBb{s}_{k}")
                b_ = xTs[k][:dcnt, :]
                if eng == "pool":
                    nc.gpsimd.tensor_mul(PB[:dcnt, 0, :], b_, b_)
                elif eng == "dve":
                    nc.vector.tensor_mul(PB[:dcnt, 0, :], b_, b_)
                else:
                    nc.scalar.activation(PB[:dcnt, 0, :], b_, AF.Square)
                PBt.append(PB)
            return PBt

        def fm_side(s, alT, xTs, PBt):
            """FM heads for side s: x = alT (aligned), b = xTs (raw).

            Pair-a = (x^2, xb), pair-b = (b^2, xb^2).  Both DR matmuls
            accumulate into the same base-0 psum rows (disjoint stationary
            columns).  AB: x-proj and b-proj accumulate into the same
            12 psum rows per nx group (TA/lin sums form in PSUM).
            """
            sk = lambda k: s * 5 + k
            PAt = []
            for k, (doff, dcnt) in enumerate(DCH):
                if k >= len(PBt):
                    PBb = fmbb.tile([128, 2, L], fp8, tag="PBb",
                                    name=f"PBb{s}_{k}")
                    b2_ = xTs[k][:dcnt, :]
                    if s == 0:
                        nc.gpsimd.tensor_mul(PBb[:dcnt, 0, :], b2_, b2_)
                    else:
                        nc.scalar.activation(PBb[:dcnt, 0, :], b2_, AF.Square)
                    PBt.append(PBb)
                PA = fma.tile([128, 2, L], fp8, tag="PAa", name=f"PAa{s}_{k}")
                PBb = PBt[k]
                x_ = alT[k][:dcnt, :]
                b_ = xTs[k][:dcnt, :]
                nc.scalar.activation(PA[:dcnt, 0, :], x_, AF.Square)
                nc.vector.tensor_mul(PA[:dcnt, 1, :], x_, b_)
                nc.vector.tensor_mul(PBb[:dcnt, 1, :], PA[:dcnt, 1, :],
                                     PA[:dcnt, 1, :])
                PAt.append(PA)
            # projections: AB nx-packed in 2 fmp tiles at 64-aligned rows
            # (V rows at pb+0:10, lin rows at pb+32:34 so PSUM reads stay
            # 32-partition aligned); pair-a/pair-b DR into 2 acc tiles
            ABt = [fmp_t(name=f"ABt{i}") for i in range(2)]
            BC = [acc_t(name=f"BC{i}") for i in range(2)]
            for k, (doff, dcnt) in enumerate(DCH):
                first, last = (k == 0), (k == 4)
                for nx in range(4):
                    nsl = ds(nx * 512, 512)
                    AB = ABt[nx // 2]
                    pb = (nx % 2) * 64
                    nc.tensor.matmul(AB[pb:pb + 34, :],
                                     pa_stat[s][k][:dcnt, 0:34],
                                     alT[k][:dcnt, nsl],
                                     start=first, stop=False,
                                     tile_position=(0, pb),
                                     skip_group_check=True)
                    nc.tensor.matmul(AB[pb:pb + 34, :],
                                     pa_stat[s][k][:dcnt, 48:82],
                                     xTs[k][:dcnt, nsl],
                                     start=False, stop=last,
                                     tile_position=(0, pb),
                                     skip_group_check=True)
                    hsl = ds((nx % 2) * 512, 512)
                    nc.tensor.matmul(BC[nx // 2][0:32, hsl],
                                     PBCv[:dcnt, sk(k), :, 0:32],
                                     PAt[k][:dcnt, :, nsl],
                                     start=first, stop=False,
                                     perf_mode=DR, skip_group_check=True)
                    nc.tensor.matmul(BC[nx // 2][0:32, hsl],
                                     PBCv[:dcnt, sk(k), :, 32:64],
                                     PBt[k][:dcnt, :, nsl],
                                     start=False, stop=last,
                                     perf_mode=DR, skip_group_check=True)
            # S assembly (all writes at 32-aligned partition starts):
            # rows 0:10 TA^2, 32:44 BC, 64:69 M^2, 96:98 lin; gap rows are
            # zeroed once so the combine matmul contracts clean zeros
            S = sp.tile([128, L], bf16, tag="S", name=f"S{s}")
            nc.gpsimd.memset(S[:], 0.0)
            for nx in range(4):
                nsl = ds(nx * 512, 512)
                AB = ABt[nx // 2]
                pb = (nx % 2) * 64
                hsl = ds((nx % 2) * 512, 512)
                nc.scalar.activation(S[0:10, nsl], AB[pb:pb + 10, :],
                                     AF.Square)
                nc.vector.tensor_copy(S[96:98, nsl], AB[pb + 32:pb + 34, :])
                nc.vector.tensor_copy(S[32:44, nsl], BC[nx // 2][0:12, hsl])
                nc.scalar.activation(S[64:69, nsl], BC[nx // 2][0:5, hsl],
                                     AF.Square)
            for nx in range(4):
                nsl = ds(nx * 512, 512)
                cps = fmp_t(name="cps")
                nc.tensor.matmul(cps[0:3, :], cb2[0:98, ds(s * 3, 3)],
                                 S[0:98, nsl], start=True, stop=True)
                o = ob.tile([3, 512], f32, tag="ob", name="o")
                if s == 0:
                    nc.vector.scalar_tensor_tensor(
                        o[:, :], cps[0:3, :], w0sb[:, s:s + 1],
                        zerob[0:3, :], op0=ALU.add, op1=ALU.add)
                else:
                    nc.scalar.activation(o[:, :], cps[0:3, :], AF.Identity,
                                         bias=w0sb[:, s:s + 1])
                nc.sync.dma_start(out_d[s, :, nsl], o[:, :])

        # ---------------- main flow ----------------
        E1 = e_tiles(e1p, "1")
        for i in range(LT):
            for h in range(NH):
                emit_e_unit(0, 1, E1, i, h)
        E2 = e_tiles(big, "2")

        alp = ctx.enter_context(tc.tile_pool(name="alp", bufs=1))
        rp = ctx.enter_context(tc.tile_pool(name="rp", bufs=1))
        fma = ctx.enter_context(tc.tile_pool(name="fma", bufs=3))
        fmbb = ctx.enter_context(tc.tile_pool(name="fmbb", bufs=5))

        # E2 units in tile-major order, interleaved into aligned1's m-loop
        e2units = [(2 * pi + j, h)
                   for pi in range(NP) for j in range(2) for h in range(NH)]

        def e2_hook(m):
            for i, h in e2units[m * 8:(m + 1) * 8]:
                emit_e_unit(1, 0, E2, i, h)

        # side-0 b^2 prebuilt on Pool (fills the E1-exp window)
        PB0 = prebuild_b2(0, xT[0], eng="pool")
        qaT = aligned_T(1, E1, "q", hook=e2_hook, r_on_act=False)
        e1_cm.__exit__(None, None, None)
        sp = ctx.enter_context(tc.tile_pool(name="sp", bufs=1))
        ob = ctx.enter_context(tc.tile_pool(name="ob", bufs=2))
        fm_side(0, qaT, xT[0], PB0)
        PB1 = prebuild_b2(1, xT[1], eng="pool")
        paT = aligned_T(0, E2, "p", psa_on_acc=True)
        fm_side(1, paT, xT[1], PB1)


def _host_prep(W1, b1, W2, b2, cat_w0, cat_w, cat_V, dm_w0, dm_w, dm_V):
    # dense pair weights: wpair[pc][k][j] = W_t rows; pc<2: d=pc*256+j*128+k
    # pc=2: j0 rows 0:64 = d 512:576, j1 rows 0:24 = d 576:600
    wpair = np.zeros((3, 128, 2, 1024), np.float32)
    for t, W in enumerate((W1, W2)):
        for pc in range(2):
            for j in range(2):
                d0 = pc * 256 + j * 128
                wpair[pc, :, j, t * U:(t + 1) * U] = W[d0:d0 + 128]
        wpair[2, 0:64, 0, t * U:(t + 1) * U] = W[512:576]
        wpair[2, 0:24, 1, t * U:(t + 1) * U] = W[576:600]

    # PA stationaries (bf16): x-cols 0:34, b-cols 48:82; x/b accumulate
    # into the same psum rows, so signs are folded here:
    # rows 0:5 TAcat = x@Va + b@Vb; 5:10 TAdiff = x@Vd - b@Vd;
    # rows 32:34 (PSUM-read-alignment): lincat = x@wcat_a + b@wcat_b and
    # lindiff = x@wd - b@wd
    pastat = np.zeros((10, 128, 96), np.float32)
    # PB/PC stationaries (fp8 pairs), same as v2
    pbcstat = np.zeros((10, 128, 2, 64), np.float32)
    for s in range(2):
        ci, di, mi = s, s, s + 2
        Va = cat_V[ci][:, :D]
        Vb = cat_V[ci][:, D:]
        Vd = dm_V[di]
        Vm = dm_V[mi]
        ua = (Va ** 2).sum(0) * USC
        ub = (Vb ** 2).sum(0) * USC
        ud = (Vd ** 2).sum(0) * USC
        um = (Vm ** 2).sum(0) * USC
        xs = np.zeros((D, 96), np.float32)
        xs[:, 0:5] = Va.T
        xs[:, 5:10] = Vd.T
        xs[:, 32] = cat_w[ci, :D]
        xs[:, 33] = dm_w[di]
        xs[:, 48 + 0:48 + 5] = Vb.T
        xs[:, 48 + 5:48 + 10] = -Vd.T
        xs[:, 48 + 32] = cat_w[ci, D:]
        xs[:, 48 + 33] = -dm_w[di]
        bs = np.zeros((D, 2, 64), np.float32)
        # pair-a = (x^2, xb): j0 -> x^2 stats, j1 -> xb stats
        bs[:, 0, 5] = ua
        bs[:, 0, 6] = ud
        bs[:, 1, 0:5] = Vm.T
        bs[:, 1, 7] = dm_w[mi]
        bs[:, 1, 8] = ud
        # pair-b = (b^2, xb^2): j0 -> b^2 stats, j1 -> xb^2 stats
        bs[:, 0, 32 + 9] = ub
        bs[:, 0, 32 + 10] = ud
        bs[:, 1, 32 + 11] = um
        for k, (doff, dcnt) in enumerate(DCH):
            pastat[s * 5 + k, :dcnt] = xs[doff:doff + dcnt]
            pbcstat[s * 5 + k, :dcnt] = bs[doff:doff + dcnt]

    # combine matrix: S rows -> 3 outputs per side
    # S rows: 0:5 TAcat^2, 5:10 TAdiff^2, 32:44 = BC rows (32+c),
    # 64:69 M^2, 96 lincat, 97 lindiff
    comb2 = np.zeros((128, 6), np.float32)
    for s in range(2):
        C = comb2[:, s * 3:(s + 1) * 3]
        C[96, 0] = 1.0            # lincat
        C[0:5, 0] = 0.5           # cat quads (TA^2 rows)
        C[32 + 5, 0] = -0.5 / USC   # x2@ua
        C[32 + 9, 0] = -0.5 / USC   # b2@ub
        C[97, 1] = 1.0            # lindiff
        C[5:10, 1] = 0.5          # diff quads (TA^2 rows)
        C[32 + 6, 1] = -0.5 / USC   # x2@ud
        C[32 + 10, 1] = -0.5 / USC  # b2@ud
        C[32 + 8, 1] = 1.0 / USC    # xb@ud
        C[32 + 7, 2] = 1.0          # xb@w_m
        C[64:69, 2] = 0.5           # mul quads (M^2 rows)
        C[32 + 11, 2] = -0.5 / USC  # xb2@um

    biasp = np.zeros((128, 6), np.float32)
    for t, b in enumerate((b1, b2)):
        for m, (uoff, ucnt) in enumerate([(0, 128), (128, 128), (256, 44)]):
            if m < 2:
                biasp[:ucnt, t * 3 + m] = b[uoff:uoff + ucnt]
            else:
                biasp[0:32, t * 3 + m] = b[256:288]
                biasp[32:44, t * 3 + m] = b[288:300]

    w0col = np.zeros((3, 2), np.float32)
    for s in range(2):
        w0col[0, s] = cat_w0[s, 0]
        w0col[1, s] = dm_w0[s, 0]
        w0col[2, s] = dm_w0[s + 2, 0]
    return wpair, pastat, pbcstat, comb2, biasp, w0col


_PROG = None


def _get_prog():
    global _PROG
    if _PROG is None:
        from concourse import bacc
        nc = bacc.Bacc(None, target_bir_lowering=False)
        _emit(nc, L_FULL)
        nc.finalize()
        _PROG = nc
    return _PROG


def _in_maps(stack_input, W1, b1, W2, b2, fm_cat_w0, fm_cat_w, fm_cat_V,
             fm_dm_w0, fm_dm_w, fm_dm_V):
    import concourse.mybir as mybir
    bf = mybir.dt.np(mybir.dt.bfloat16)
    f8 = mybir.dt.np(mybir.dt.float8e4)
    f = lambda a: np.ascontiguousarray(np.asarray(a, np.float32))
    stack_input = f(stack_input)
    wpair, pastat, pbcstat, comb2, biasp, w0col = _host_prep(
        f(W1), f(b1), f(W2), f(b2), f(fm_cat_w0), f(fm_cat_w), f(fm_cat_V),
        f(fm_dm_w0), f(fm_dm_w), f(fm_dm_V))
    common = {
        "wpair8": np.ascontiguousarray(wpair.astype(f8)),
        "pastat": np.ascontiguousarray(pastat.astype(bf)),
        "pbcstat": np.ascontiguousarray(pbcstat.astype(f8)),
        "comb2": np.ascontiguousarray(comb2.astype(bf)),
        "biasp": biasp, "w0col": w0col,
    }
    L = L_FULL
    NP = L // 256
    xb_all = stack_input.astype(bf)      # [2, B, L, D]
    maps = []
    for b in range(N_CORES):
        xb = xb_all[:, b]                # [2, L, D] bf16
        xTf = np.zeros((2, 5, 128, L), bf)
        for t in range(2):
            xtT = xb[t].T                # [D, L]
            for k, (doff, dcnt) in enumerate(DCH):
                xTf[t, k, :dcnt] = xtT[doff:doff + dcnt]
        x8 = np.zeros((2, 3, 128, 2, L), f8)
        for t in range(2):
            x8[t, 0, :, 0] = xTf[t, 0].astype(f8)
            x8[t, 0, :, 1] = xTf[t, 1].astype(f8)
            x8[t, 1, :, 0] = xTf[t, 2].astype(f8)
            x8[t, 1, :, 1] = xTf[t, 3].astype(f8)
            x8[t, 2, 0:64, 0] = xTf[t, 4, 0:64].astype(f8)
            x8[t, 2, 0:24, 1] = xTf[t, 4, 64:88].astype(f8)
        natp = np.zeros((2, NP, 128, 2, NATW), f8)
        for t in range(2):
            xn = xb[t].reshape(NP, 2, 128, D).transpose(0, 2, 1, 3)
            natp[t, :, :, :, 0:512] = xn[:, :, :, 0:512].astype(f8)
            natp[t, :, :, :, 512:512 + 88] = xn[:, :, :, 512:600].astype(f8)
            natp[t, :, :, :, 512 + ONES_COL] = 1.0
        maps.append(dict(common,
                         x8=np.ascontiguousarray(x8),
                         xt=np.ascontiguousarray(xTf),
                         natp=np.ascontiguousarray(natp)))
    return maps


def kernel(stack_input, W1, b1, W2, b2, fm_cat_w0, fm_cat_w, fm_cat_V,
           fm_dm_w0, fm_dm_w, fm_dm_V):
    from concourse.bass_utils import run_bass_kernel_spmd

    in_maps = _in_maps(stack_input, W1, b1, W2, b2, fm_cat_w0, fm_cat_w,
                       fm_cat_V, fm_dm_w0, fm_dm_w, fm_dm_V)
    nc = _get_prog()
    res = run_bass_kernel_spmd(nc, in_maps, core_ids=list(range(N_CORES)))
    outs = [r["out"] for r in res.results]            # each [2, 3, L]
    fp = np.stack([o[0].T for o in outs]).astype(np.float32)   # [8, L, 3]
    fq = np.stack([o[1].T for o in outs]).astype(np.float32)
    return fp, fq
